# revision 42
# baseline (speedup 1.0000x reference)
"""MultiHeadAttention TRN2 Bass kernel (B=2, S=2048, D=1024, H=16, d=64).

Sharding: 8 cores = 2 (batch) x 4 (head groups of 4 heads).
Each core computes, for its batch b and head slice hs (256 dims):
    K^T = (Wk[hs,:] @ x_k^T + bk)    [256, 2048]   (dh on partitions)
    Q^T likewise; V = x_v @ Wv[hs,:].T + bv        [2048, 256]  (s on partitions)
    per head pair (2m, 2m+1): S^T = K_h @ Q_h^T, the two heads' score
    matmuls occupy disjoint PE row groups (contraction 64 at partitions
    0-63 / 64-127) and different PSUM banks -> they stream concurrently.
    P^T = exp(S^T / 8)   (scores ~ N(0,1), exp is safe without max-sub)
    [O^T ; denom] = [V_h | 1]^T @ P^T   (ones column folds the softmax
                                         denominator into the PV matmul)
    O^T = O^T * (1/denom)  (reciprocal_approx_fast, PE K=1 replicate)
    y_partial = O^T.T @ Wo[:, hs].T     [2048, 1024]
Host: y[b] = sum of 4 head-group partials + bo.

Everything the PE streams is bf16 (host-side cast: halves DMA, removes
all DVE casts, 1 cycle/row matmuls). The schedule is paced by the two
hard floors: Scalar-engine exp over 16.8M score elements (~137us) and
PE matmul rows (~110us). x is DMA'd in [128,512] column chunks through
rotating pools so the first score matmul lands ~10us in; after that the
emission keeps Scalar saturated: per sk-tile the PE emits scores(k)
BEFORE pv(k-1) (software pipeline, so the PE never blocks on the exp it
is feeding), and projection/output-projection work is woven into the
per-pair PE slack.
"""

import numpy as np
import ml_dtypes

import concourse.bass as bass
import concourse.tile as tile
import concourse.mybir as mybir
from concourse import bacc
from concourse.bass_utils import run_bass_kernel_spmd

D_MODEL = 1024
NUM_HEADS = 16
HEAD_DIM = 64
B, S = 2, 2048
N_CORES = 8
HG = 4                  # head-groups
HEADS_PER_CORE = NUM_HEADS // HG        # 4
DH = HEADS_PER_CORE * HEAD_DIM          # 256 output dims per core
KT = D_MODEL // 128                     # 8 contraction tiles
ST = S // 128                           # 16 sequence tiles
SB = S // 512                           # 4 sequence blocks of 512

F32 = mybir.dt.float32
F32R = mybir.dt.float32r
BF16 = mybir.dt.bfloat16
AF = mybir.ActivationFunctionType
BF16_NP = ml_dtypes.bfloat16

_cached_nc = None


def build_nc():
    nc = bacc.Bacc("TRN2", target_bir_lowering=False, debug=False)

    xq_t = nc.declare_dram_parameter("xq_t", [D_MODEL, S], BF16, isOutput=False)
    xk_t = nc.declare_dram_parameter("xk_t", [D_MODEL, S], BF16, isOutput=False)
    xv_t = nc.declare_dram_parameter("xv_t", [D_MODEL, S], BF16, isOutput=False)
    wq_t = nc.declare_dram_parameter("wq_t", [128, KT * DH], BF16, isOutput=False)
    wk_t = nc.declare_dram_parameter("wk_t", [128, KT * DH], BF16, isOutput=False)
    wv_t = nc.declare_dram_parameter("wv_t", [128, KT * DH], BF16, isOutput=False)
    wo_t = nc.declare_dram_parameter("wo_t", [128, 2 * D_MODEL], BF16, isOutput=False)
    bqk = nc.declare_dram_parameter("bqk", [128, 4], F32, isOutput=False)
    bv = nc.declare_dram_parameter("bv", [1, DH], BF16, isOutput=False)
    y = nc.declare_dram_parameter("y", [S, D_MODEL], F32, isOutput=True)

    with tile.TileContext(nc) as tc:
        _emit(nc, tc, xq_t, xk_t, xv_t, wq_t, wk_t, wv_t, wo_t, bqk, bv, y)
    nc.compile()
    return nc


def _emit(nc, tc, xq_t, xk_t, xv_t, wq_t, wk_t, wv_t, wo_t, bqk, bv, y):
    from contextlib import ExitStack

    ctx = ExitStack()
    with ctx:
        # ---- persistent tiles -------------------------------------------
        persist = ctx.enter_context(tc.tile_pool(name="persist", bufs=1))
        qt = [persist.tile([128, S], BF16, tag=f"qt{m}", name=f"qt{m}")
              for m in range(2)]
        kt_sb = [persist.tile([128, S], BF16, tag=f"kt{m}", name=f"kt{m}")
                 for m in range(2)]
        v_sb = [persist.tile([128, HEADS_PER_CORE * 65], BF16, tag=f"v{i}",
                             name=f"v{i}") for i in range(ST)]
        ot = [persist.tile([128, S], BF16, tag=f"ot{m}", name=f"ot{m}")
              for m in range(2)]
        wo_flat = persist.tile([128, 2 * D_MODEL], BF16, tag="wof", name="wof")
        wo_r = [wo_flat[:, m * D_MODEL:(m + 1) * D_MODEL] for m in range(2)]
        ones_row = persist.tile([1, S], BF16, tag="ones")
        ones64 = persist.tile([33, 64], F32, tag="ones64")
        ones64_r = persist.tile([33, 64], F32R, tag="ones64r")
        ones_col = persist.tile([128, HEADS_PER_CORE], F32, tag="onesc")
        bqk_c = persist.tile([128, 4], F32, tag="bqk")  # bq|bk per-partition
        bq_c, bk_c = bqk_c[:, 0:2], bqk_c[:, 2:4]
        bv_r = persist.tile([1, DH], BF16, tag="bvr")
        w_flat = {n: persist.tile([128, KT * DH], BF16, tag=f"w{n}",
                                  name=f"w{n}") for n in ("k", "q", "v")}
        w_sb = {n: [w_flat[n][:, k * DH:(k + 1) * DH] for k in range(KT)]
                for n in ("k", "q", "v")}

        # ---- x pools: fast-start [128,512] chunks + bulk remainder ------
        xk_pool = ctx.enter_context(tc.tile_pool(name="xk", bufs=8))
        xkb_pool = ctx.enter_context(tc.tile_pool(name="xkb", bufs=8))
        xq_pool = ctx.enter_context(tc.tile_pool(name="xq", bufs=8))
        xqb_pool = ctx.enter_context(tc.tile_pool(name="xqb", bufs=8))
        xv_pool = ctx.enter_context(tc.tile_pool(name="xv", bufs=8))
        xc_store = {}

        def dma_chunk1(pool, key, dram, k, nb):
            t = pool.tile([128, 512], BF16, tag="xc", name="xc")
            nc.sync.dma_start(
                t[:], dram[k * 128:(k + 1) * 128, nb * 512:(nb + 1) * 512])
            xc_store[(key, k, nb)] = (t, 0)

        def dma_chunks(pool, key, dram, nb):
            for k in range(KT):
                dma_chunk1(pool, key, dram, k, nb)

        def dma_bulk(pool, key, dram, nb0, nbn, tag):
            w = (nbn - nb0) * 512
            for k in range(KT):
                t = pool.tile([128, w], BF16, tag=tag, name="xb")
                nc.sync.dma_start(
                    t[:], dram[k * 128:(k + 1) * 128,
                               nb0 * 512:nbn * 512])
                for nb in range(nb0, nbn):
                    xc_store[(key, k, nb)] = (t, (nb - nb0) * 512)

        def xs(key, k, nb):
            t, off = xc_store[(key, k, nb)]
            return t[:, off:off + 512]

        # DMA priority order == consumption order
        nc.sync.dma_start(bqk_c[:], bqk[:, :])
        nc.sync.dma_start(w_flat["k"][:], wk_t[:, :])
        for k in range(KT):
            dma_chunk1(xk_pool, "k", xk_t, k, 0)
            dma_chunk1(xq_pool, "q", xq_t, k, 0)
        nc.sync.dma_start(w_flat["q"][:], wq_t[:, :])
        dma_bulk(xkb_pool, "k", xk_t, 1, 4, "xkb")
        dma_bulk(xqb_pool, "q", xq_t, 1, 4, "xqb")
        nc.sync.dma_start(w_flat["v"][:], wv_t[:, :])
        nc.sync.dma_start(bv_r[:], bv[:])
        dma_bulk(xv_pool, "v", xv_t, 0, 4, "xvb")
        nc.sync.dma_start(wo_flat[:], wo_t[:, :])

        # ---- pipelined-body pools ---------------------------------------
        ps_s = ctx.enter_context(
            tc.tile_pool(name="pss", bufs=2, space="PSUM"))      # 4 banks
        ps_acc = ctx.enter_context(
            tc.tile_pool(name="psacc", bufs=1, space="PSUM"))    # 2 banks
        ps_w = ctx.enter_context(
            tc.tile_pool(name="psw", bufs=2, space="PSUM"))      # 2 banks
        pt_pool = ctx.enter_context(tc.tile_pool(name="pt", bufs=19))
        sm_pool = ctx.enter_context(tc.tile_pool(name="small", bufs=1))
        sm2_pool = ctx.enter_context(tc.tile_pool(name="small2", bufs=2))
        y_pool = ctx.enter_context(tc.tile_pool(name="ysb", bufs=2))

        # constants
        nc.gpsimd.memset(ones_row[:], 1.0)
        nc.vector.memset(ones64[:], 1.0)
        nc.vector.tensor_copy(ones64_r[:], ones64[:])
        nc.vector.memset(ones_col[:], 1.0)

        # ---- building blocks --------------------------------------------
        def proj_qk_m(name, dst, bias_c, nb, m):
            """Project one (512-col, m-half) block of Q^T or K^T."""
            ps = ps_w.tile([128, 512], F32, tag="pw", name="pw")
            for k in range(KT):
                nc.tensor.matmul(
                    ps[:],
                    w_sb[name][k][:, m * 128:(m + 1) * 128],
                    xs(name, k, nb),
                    start=(k == 0), stop=(k == KT - 1),
                )
            nc.scalar.activation(
                dst[m][:, nb * 512:(nb + 1) * 512], ps[:],
                AF.Identity, bias=bias_c[:, m:m + 1])

        def v_chunk(i):
            """Project V for s-tile i into v_sb[i] (+ ones column). One
            accumulation group per PSUM tile: interleaved groups sharing a
            bank clobber each other's has_written state."""
            nb, col = divmod(i, 4)
            ps = ps_w.tile([128, 512], F32, tag="pw", name="pw")
            for k in range(KT):
                nc.tensor.matmul(
                    ps[:, 0:256],
                    xs("v", k, nb)[:, col * 128:(col + 1) * 128],
                    w_sb["v"][k][:],
                    start=(k == 0), stop=False,
                )
            nc.tensor.matmul(
                ps[:, 0:256],
                ones_row[0:1, i * 128:(i + 1) * 128],
                bv_r[0:1, :],
                start=False, stop=True,
            )
            src = ps[:, 0:256].rearrange("p (h c) -> p h c", c=64)
            vv = v_sb[i].rearrange("p (h c) -> p h c", c=65)
            nc.vector.tensor_copy(vv[:, :, 0:64], src)
            nc.vector.tensor_copy(vv[:, :, 64], ones_col[:])

        def scores(qb, m, k):
            """Score pair (heads 2m,2m+1), sk-tile k, sq-block qb. The two
            K=64 matmuls use disjoint PE row groups + PSUM banks and stream
            concurrently. Returns the exp'd bf16 tile."""
            ss = ps_s.tile([128, 1024], F32, tag="ss", name="ss")
            for p2 in range(2):
                po = 64 * p2
                nc.tensor.matmul(
                    ss[:, p2 * 512:(p2 + 1) * 512],
                    kt_sb[m][po:po + 64, k * 128:(k + 1) * 128],
                    qt[m][po:po + 64, qb * 512:(qb + 1) * 512],
                    start=True, stop=True,
                )
            pt = pt_pool.tile([128, 1024], BF16, tag="pt", name="pt")
            nc.scalar.activation(
                pt[:], ss[:], AF.Exp, scale=1.0 / float(np.sqrt(HEAD_DIM)))
            return pt

        def pv(m, k, pt, accs):
            for p2 in range(2):
                h = 2 * m + p2
                nc.tensor.matmul(
                    accs[p2][:],
                    v_sb[k][:, h * 65:(h + 1) * 65],
                    pt[:, p2 * 512:(p2 + 1) * 512],
                    start=(k == 0), stop=(k == ST - 1),
                )

        def norm_stage1(accs):
            """Evict O rows + denominators to SBUF (frees the PSUM accs for
            the next pair immediately) and start the batched reciprocal."""
            o_sb = []
            den2 = sm_pool.tile([33, 512], F32, tag="den2", name="den2")
            for p2 in range(2):
                o = sm2_pool.tile([64, 512], BF16, tag=f"o{p2}", name="osb")
                nc.vector.tensor_copy(o[:], accs[p2][0:64, :])
                o_sb.append(o)
                nc.vector.tensor_copy(den2[32 * p2:32 * p2 + 1, :],
                                      accs[p2][64:65, :])
            recip2 = sm2_pool.tile([33, 512], F32R, tag="recip2", name="recip2")
            with nc.allow_low_precision(reason="softmax denom"):
                nc.vector.reciprocal(recip2[:], den2[:])
            return (o_sb, recip2)

        def norm_apply(qb, m, st):
            """ot[m][:, qb block] = O^T * recip: PE K=1 replicate + GpSimd
            multiply (SBUF-only operands, keeps DVE free)."""
            o_sb, recip2 = st
            for p2 in range(2):
                rep = ps_w.tile([128, 512], F32, tag="pw", name="pw")
                nc.tensor.matmul(
                    rep[0:64, :], ones64_r[32 * p2:32 * p2 + 1, :],
                    recip2[32 * p2:32 * p2 + 1, :],
                    start=True, stop=True,
                )
                rep_sb = sm_pool.tile([64, 512], BF16, tag="repsb",
                                      name="repsb")
                nc.vector.tensor_copy(rep_sb[:], rep[0:64, :])
                po = 64 * p2
                nc.gpsimd.tensor_mul(
                    ot[m][po:po + 64, qb * 512:(qb + 1) * 512],
                    o_sb[p2][:], rep_sb[:])

        def yproj_i(i, ysb_holder):
            """Output projection for s-tile i; DMA when both halves done."""
            if ysb_holder[0] is None:
                ysb_holder[0] = y_pool.tile([128, D_MODEL], F32, tag="ysb", name="ysb")
            ysb = ysb_holder[0]
            for nb2 in range(2):
                ps = ps_w.tile([128, 512], F32, tag="pw", name="pw")
                for m in range(2):
                    nc.tensor.matmul(
                        ps[:],
                        ot[m][:, i * 128:(i + 1) * 128],
                        wo_r[m][:, nb2 * 512:(nb2 + 1) * 512],
                        start=(m == 0), stop=(m == 1),
                    )
                nc.vector.tensor_copy(
                    ysb[:, nb2 * 512:(nb2 + 1) * 512], ps[:])
            nc.sync.dma_start(y[i * 128:(i + 1) * 128, :], ysb[:])
            ysb_holder[0] = None

        # =============== emission schedule ===============================
        pairs = [(qb, m) for qb in range(SB) for m in range(2)]

        # lead-in: K block 0 (both halves), Q block 0
        for m in range(2):
            proj_qk_m("k", kt_sb, bk_c, 0, m)
        for m in range(2):
            proj_qk_m("q", qt, bq_c, 0, m)

        # p0: scores of pair (0,0); K blocks 1-3 + Q block 1 as PE filler
        pts_prev = []
        for k in range(ST):
            pts_prev.append(scores(0, 0, k))
            if k in (2, 4, 6):          # K blocks 1..3
                nb = k // 2
                for m in range(2):
                    proj_qk_m("k", kt_sb, bk_c, nb, m)
            elif k in (9, 12):          # Q block 1
                proj_qk_m("q", qt, bq_c, 1, 0 if k == 9 else 1)

        # windows p1..p7: scores of pair p run while the PREVIOUS pair's PV
        # drains (its pts are held; one pair of lag), then norm(prev).
        # p1 additionally weaves in the 16 V-projection chunks; later
        # windows weave in Q blocks 2-3 and the output projection.
        def fillers_for(p, k, yh):
            qb = pairs[p][0]
            if p in (2, 4) and k in (5, 11):      # Q blocks 2,3
                proj_qk_m("q", qt, bq_c, qb + 1, 0 if k == 5 else 1)
            elif p in (3, 5, 7) and k >= 6 and k % 3 == 0:
                # yproj of qb-1 (4 s-tiles at k=6,9,12,15, after norm_apply)
                yproj_i((qb - 1) * 4 + (k - 6) // 3, yh)

        yh = [None]
        prev_pair = (0, 0)
        pending_norm = None      # (qb, m, stage1 state) awaiting norm_apply
        last = len(pairs) - 1
        for p in range(1, len(pairs)):
            qb, m = pairs[p]
            accs_run = [ps_acc.tile([65, 512], F32, tag=f"acc{pp}",
                                    name=f"acc{pp}") for pp in range(2)]
            pts_cur = []
            for k in range(ST):
                pts_cur.append(scores(qb, m, k))
                if k == 5 and pending_norm is not None:
                    norm_apply(*pending_norm)
                    pending_norm = None
                if p == 1:
                    v_chunk(k)
                # prev pair's PV: two-behind so the exp it reads is long done;
                # in the final window, front-load two per iter to finish early
                if p < last:
                    if k >= 2:
                        pv(prev_pair[1], k - 2, pts_prev[k - 2], accs_run)
                elif k < 8:
                    pv(prev_pair[1], 2 * k, pts_prev[2 * k], accs_run)
                    pv(prev_pair[1], 2 * k + 1, pts_prev[2 * k + 1], accs_run)
                fillers_for(p, k, yh)
                if p == last and k == 8:
                    st = norm_stage1(accs_run)
                    pending_norm = (prev_pair[0], prev_pair[1], st)
            if p < last:
                pv(prev_pair[1], 14, pts_prev[14], accs_run)
                pv(prev_pair[1], 15, pts_prev[15], accs_run)
                st = norm_stage1(accs_run)
                pending_norm = (prev_pair[0], prev_pair[1], st)
                prev_pair, pts_prev = (qb, m), pts_cur
            else:
                # final window: prev (P6) fully drained and staged at k==8;
                # start the last pair's own PV in the back half
                accs_last = [ps_acc.tile([65, 512], F32, tag=f"acc{pp}",
                                         name=f"acc{pp}") for pp in range(2)]
                prev_pair, pts_prev = (qb, m), pts_cur
        for k in range(8):
            pv(prev_pair[1], k, pts_prev[k], accs_last)
        if pending_norm is not None:
            norm_apply(*pending_norm)
            pending_norm = None
        for k in range(8, ST):
            pv(prev_pair[1], k, pts_prev[k], accs_last)
        st = norm_stage1(accs_last)
        norm_apply(prev_pair[0], prev_pair[1], st)
        for i4 in range(4):
            yproj_i(3 * 4 + i4, yh)


def _get_nc():
    global _cached_nc
    if _cached_nc is None:
        _cached_nc = build_nc()
    return _cached_nc


def _make_in_maps(query, key, value, Wq, bq, Wk, bk, Wv, bv, Wo):
    """Shard + transpose + bf16-cast on host: core c = (b, hg), b = c // HG."""
    query = np.asarray(query, dtype=np.float32)
    key = np.asarray(key, dtype=np.float32)
    value = np.asarray(value, dtype=np.float32)
    Wq, Wk, Wv, Wo = (np.asarray(w, dtype=np.float32) for w in (Wq, Wk, Wv, Wo))
    bq, bk, bv = (np.asarray(b_, dtype=np.float32) for b_ in (bq, bk, bv))
    in_maps = []
    xq_t = [np.ascontiguousarray(query[b].T).astype(BF16_NP) for b in range(B)]
    xk_t = [np.ascontiguousarray(key[b].T).astype(BF16_NP) for b in range(B)]
    xv_t = [np.ascontiguousarray(value[b].T).astype(BF16_NP) for b in range(B)]
    def tile_w(WT):          # [1024, 256] -> [128, 8*256] (k-tiles packed)
        return np.ascontiguousarray(
            WT.reshape(KT, 128, DH).transpose(1, 0, 2).reshape(128, KT * DH)
        ).astype(BF16_NP)

    for c in range(N_CORES):
        b, hg = divmod(c, HG)
        hs = slice(hg * DH, (hg + 1) * DH)
        wo_tiled = np.ascontiguousarray(
            Wo[:, hs].T.reshape(2, 128, D_MODEL).transpose(1, 0, 2)
            .reshape(128, 2 * D_MODEL)).astype(BF16_NP)
        bqk_pack = np.concatenate(
            [bq[hs].reshape(2, 128).T, bk[hs].reshape(2, 128).T],
            axis=1)          # [128, 4] = bq cols | bk cols
        in_maps.append({
            "xq_t": xq_t[b],
            "xk_t": xk_t[b],
            "xv_t": xv_t[b],
            "wq_t": tile_w(Wq[hs, :].T),
            "wk_t": tile_w(Wk[hs, :].T),
            "wv_t": tile_w(Wv[hs, :].T),
            "wo_t": wo_tiled,
            "bqk": np.ascontiguousarray(bqk_pack),
            "bv": np.ascontiguousarray(bv[hs]).reshape(1, DH).astype(BF16_NP),
        })
    return in_maps


def run(inputs, trace=False, **spmd_kwargs):
    nc = _get_nc()
    in_maps = _make_in_maps(
        inputs["query"], inputs["key"], inputs["value"],
        inputs["Wq"], inputs["bq"], inputs["Wk"], inputs["bk"],
        inputs["Wv"], inputs["bv"], inputs["Wo"])
    res = run_bass_kernel_spmd(
        nc, in_maps, list(range(N_CORES)), trace=trace, **spmd_kwargs)
    bo = np.asarray(inputs["bo"], dtype=np.float32)
    out = np.empty((B, S, D_MODEL), dtype=np.float32)
    for b in range(B):
        acc = np.zeros((S, D_MODEL), dtype=np.float32)
        for hg in range(HG):
            acc += res.results[b * HG + hg]["y"]
        out[b] = acc + bo
    return out, res


def kernel(**inputs) -> np.ndarray:
    out, _ = run(inputs, trace=False)
    return out


# revision 43
# speedup vs baseline: 1.0011x; 1.0011x over previous
"""MultiHeadAttention TRN2 Bass kernel (B=2, S=2048, D=1024, H=16, d=64).

Sharding: 8 cores = 2 (batch) x 4 (head groups of 4 heads).
Each core computes, for its batch b and head slice hs (256 dims):
    K^T = (Wk[hs,:] @ x_k^T + bk)    [256, 2048]   (dh on partitions)
    Q^T likewise; V = x_v @ Wv[hs,:].T + bv        [2048, 256]  (s on partitions)
    per head pair (2m, 2m+1): S^T = K_h @ Q_h^T, the two heads' score
    matmuls occupy disjoint PE row groups (contraction 64 at partitions
    0-63 / 64-127) and different PSUM banks -> they stream concurrently.
    P^T = exp(S^T / 8)   (scores ~ N(0,1), exp is safe without max-sub)
    [O^T ; denom] = [V_h | 1]^T @ P^T   (ones column folds the softmax
                                         denominator into the PV matmul)
    O^T = O^T * (1/denom)  (reciprocal_approx_fast, PE K=1 replicate)
    y_partial = O^T.T @ Wo[:, hs].T     [2048, 1024]
Host: y[b] = sum of 4 head-group partials + bo.

Everything the PE streams is bf16 (host-side cast: halves DMA, removes
all DVE casts, 1 cycle/row matmuls). The schedule is paced by the two
hard floors: Scalar-engine exp over 16.8M score elements (~137us) and
PE matmul rows (~110us). x is DMA'd in [128,512] column chunks through
rotating pools so the first score matmul lands ~10us in; after that the
emission keeps Scalar saturated: per sk-tile the PE emits scores(k)
BEFORE pv(k-1) (software pipeline, so the PE never blocks on the exp it
is feeding), and projection/output-projection work is woven into the
per-pair PE slack.
"""

import numpy as np
import ml_dtypes

import concourse.bass as bass
import concourse.tile as tile
import concourse.mybir as mybir
from concourse import bacc
from concourse.bass_utils import run_bass_kernel_spmd

D_MODEL = 1024
NUM_HEADS = 16
HEAD_DIM = 64
B, S = 2, 2048
N_CORES = 8
HG = 4                  # head-groups
HEADS_PER_CORE = NUM_HEADS // HG        # 4
DH = HEADS_PER_CORE * HEAD_DIM          # 256 output dims per core
KT = D_MODEL // 128                     # 8 contraction tiles
ST = S // 128                           # 16 sequence tiles
SB = S // 512                           # 4 sequence blocks of 512

F32 = mybir.dt.float32
F32R = mybir.dt.float32r
BF16 = mybir.dt.bfloat16
AF = mybir.ActivationFunctionType
BF16_NP = ml_dtypes.bfloat16

_cached_nc = None


def build_nc():
    nc = bacc.Bacc("TRN2", target_bir_lowering=False, debug=False)

    xq_t = nc.declare_dram_parameter("xq_t", [D_MODEL, S], BF16, isOutput=False)
    xk_t = nc.declare_dram_parameter("xk_t", [D_MODEL, S], BF16, isOutput=False)
    xv_t = nc.declare_dram_parameter("xv_t", [D_MODEL, S], BF16, isOutput=False)
    wq_t = nc.declare_dram_parameter("wq_t", [128, KT * DH], BF16, isOutput=False)
    wk_t = nc.declare_dram_parameter("wk_t", [128, KT * DH], BF16, isOutput=False)
    wv_t = nc.declare_dram_parameter("wv_t", [128, KT * DH], BF16, isOutput=False)
    wo_t = nc.declare_dram_parameter("wo_t", [128, 2 * D_MODEL], BF16, isOutput=False)
    bqk = nc.declare_dram_parameter("bqk", [128, 4], F32, isOutput=False)
    bv = nc.declare_dram_parameter("bv", [1, DH], BF16, isOutput=False)
    y = nc.declare_dram_parameter("y", [S, D_MODEL], F32, isOutput=True)

    with tile.TileContext(nc) as tc:
        _emit(nc, tc, xq_t, xk_t, xv_t, wq_t, wk_t, wv_t, wo_t, bqk, bv, y)
    nc.compile()
    return nc


def _emit(nc, tc, xq_t, xk_t, xv_t, wq_t, wk_t, wv_t, wo_t, bqk, bv, y):
    from contextlib import ExitStack

    ctx = ExitStack()
    with ctx:
        # ---- persistent tiles -------------------------------------------
        persist = ctx.enter_context(tc.tile_pool(name="persist", bufs=1))
        qt = [persist.tile([128, S], BF16, tag=f"qt{m}", name=f"qt{m}")
              for m in range(2)]
        kt_sb = [persist.tile([128, S], BF16, tag=f"kt{m}", name=f"kt{m}")
                 for m in range(2)]
        v_sb = [persist.tile([128, HEADS_PER_CORE * 65], BF16, tag=f"v{i}",
                             name=f"v{i}") for i in range(ST)]
        ot = [persist.tile([128, S], BF16, tag=f"ot{m}", name=f"ot{m}")
              for m in range(2)]
        wo_flat = persist.tile([128, 2 * D_MODEL], BF16, tag="wof", name="wof")
        wo_r = [wo_flat[:, m * D_MODEL:(m + 1) * D_MODEL] for m in range(2)]
        ones_row = persist.tile([1, S], BF16, tag="ones")
        ones64 = persist.tile([33, 64], F32, tag="ones64")
        ones64_r = persist.tile([33, 64], F32R, tag="ones64r")
        ones_col = persist.tile([128, HEADS_PER_CORE], F32, tag="onesc")
        bqk_c = persist.tile([128, 4], F32, tag="bqk")  # bq|bk per-partition
        bq_c, bk_c = bqk_c[:, 0:2], bqk_c[:, 2:4]
        bv_r = persist.tile([1, DH], BF16, tag="bvr")
        w_flat = {n: persist.tile([128, KT * DH], BF16, tag=f"w{n}",
                                  name=f"w{n}") for n in ("k", "q", "v")}
        w_sb = {n: [w_flat[n][:, k * DH:(k + 1) * DH] for k in range(KT)]
                for n in ("k", "q", "v")}

        # ---- x pools: fast-start [128,512] chunks + bulk remainder ------
        xk_pool = ctx.enter_context(tc.tile_pool(name="xk", bufs=8))
        xkb_pool = ctx.enter_context(tc.tile_pool(name="xkb", bufs=8))
        xq_pool = ctx.enter_context(tc.tile_pool(name="xq", bufs=8))
        xqb_pool = ctx.enter_context(tc.tile_pool(name="xqb", bufs=8))
        xv_pool = ctx.enter_context(tc.tile_pool(name="xv", bufs=8))
        xc_store = {}

        def dma_chunk1(pool, key, dram, k, nb):
            t = pool.tile([128, 512], BF16, tag="xc", name="xc")
            nc.sync.dma_start(
                t[:], dram[k * 128:(k + 1) * 128, nb * 512:(nb + 1) * 512])
            xc_store[(key, k, nb)] = (t, 0)

        def dma_chunks(pool, key, dram, nb):
            for k in range(KT):
                dma_chunk1(pool, key, dram, k, nb)

        def dma_bulk(pool, key, dram, nb0, nbn, tag):
            w = (nbn - nb0) * 512
            for k in range(KT):
                t = pool.tile([128, w], BF16, tag=tag, name="xb")
                nc.sync.dma_start(
                    t[:], dram[k * 128:(k + 1) * 128,
                               nb0 * 512:nbn * 512])
                for nb in range(nb0, nbn):
                    xc_store[(key, k, nb)] = (t, (nb - nb0) * 512)

        def xs(key, k, nb):
            t, off = xc_store[(key, k, nb)]
            return t[:, off:off + 512]

        # DMA priority order == consumption order
        nc.sync.dma_start(bqk_c[:], bqk[:, :])
        nc.sync.dma_start(w_flat["k"][:], wk_t[:, :])
        for k in range(KT):
            dma_chunk1(xk_pool, "k", xk_t, k, 0)
            dma_chunk1(xq_pool, "q", xq_t, k, 0)
        nc.sync.dma_start(w_flat["q"][:], wq_t[:, :])
        dma_bulk(xkb_pool, "k", xk_t, 1, 4, "xkb")
        dma_bulk(xqb_pool, "q", xq_t, 1, 4, "xqb")
        nc.sync.dma_start(w_flat["v"][:], wv_t[:, :])
        nc.sync.dma_start(bv_r[:], bv[:])
        dma_bulk(xv_pool, "v", xv_t, 0, 4, "xvb")
        nc.sync.dma_start(wo_flat[:], wo_t[:, :])

        # ---- pipelined-body pools ---------------------------------------
        ps_s = ctx.enter_context(
            tc.tile_pool(name="pss", bufs=2, space="PSUM"))      # 4 banks
        ps_acc = ctx.enter_context(
            tc.tile_pool(name="psacc", bufs=1, space="PSUM"))    # 2 banks
        ps_w = ctx.enter_context(
            tc.tile_pool(name="psw", bufs=2, space="PSUM"))      # 2 banks
        pt_pool = ctx.enter_context(tc.tile_pool(name="pt", bufs=19))
        sm_pool = ctx.enter_context(tc.tile_pool(name="small", bufs=1))
        sm2_pool = ctx.enter_context(tc.tile_pool(name="small2", bufs=2))
        y_pool = ctx.enter_context(tc.tile_pool(name="ysb", bufs=2))

        # constants
        nc.gpsimd.memset(ones_row[:], 1.0)
        nc.vector.memset(ones64[:], 1.0)
        nc.vector.tensor_copy(ones64_r[:], ones64[:])
        nc.vector.memset(ones_col[:], 1.0)

        # ---- building blocks --------------------------------------------
        def proj_qk_m(name, dst, bias_c, nb, m):
            """Project one (512-col, m-half) block of Q^T or K^T."""
            ps = ps_w.tile([128, 512], F32, tag="pw", name="pw")
            for k in range(KT):
                nc.tensor.matmul(
                    ps[:],
                    w_sb[name][k][:, m * 128:(m + 1) * 128],
                    xs(name, k, nb),
                    start=(k == 0), stop=(k == KT - 1),
                )
            nc.scalar.activation(
                dst[m][:, nb * 512:(nb + 1) * 512], ps[:],
                AF.Identity, bias=bias_c[:, m:m + 1])

        def v_chunk(i):
            """Project V for s-tile i into v_sb[i] (+ ones column). One
            accumulation group per PSUM tile: interleaved groups sharing a
            bank clobber each other's has_written state."""
            nb, col = divmod(i, 4)
            ps = ps_w.tile([128, 512], F32, tag="pw", name="pw")
            for k in range(KT):
                nc.tensor.matmul(
                    ps[:, 0:256],
                    xs("v", k, nb)[:, col * 128:(col + 1) * 128],
                    w_sb["v"][k][:],
                    start=(k == 0), stop=False,
                )
            nc.tensor.matmul(
                ps[:, 0:256],
                ones_row[0:1, i * 128:(i + 1) * 128],
                bv_r[0:1, :],
                start=False, stop=True,
            )
            src = ps[:, 0:256].rearrange("p (h c) -> p h c", c=64)
            vv = v_sb[i].rearrange("p (h c) -> p h c", c=65)
            nc.vector.tensor_copy(vv[:, :, 0:64], src)
            nc.vector.tensor_copy(vv[:, :, 64], ones_col[:])

        def scores(qb, m, k):
            """Score pair (heads 2m,2m+1), sk-tile k, sq-block qb. The two
            K=64 matmuls use disjoint PE row groups + PSUM banks and stream
            concurrently. Returns the exp'd bf16 tile."""
            ss = ps_s.tile([128, 1024], F32, tag="ss", name="ss")
            for p2 in range(2):
                po = 64 * p2
                nc.tensor.matmul(
                    ss[:, p2 * 512:(p2 + 1) * 512],
                    kt_sb[m][po:po + 64, k * 128:(k + 1) * 128],
                    qt[m][po:po + 64, qb * 512:(qb + 1) * 512],
                    start=True, stop=True,
                )
            pt = pt_pool.tile([128, 1024], BF16, tag="pt", name="pt")
            nc.scalar.activation(
                pt[:], ss[:], AF.Exp, scale=1.0 / float(np.sqrt(HEAD_DIM)))
            return pt

        def pv(m, k, pt, accs):
            for p2 in range(2):
                h = 2 * m + p2
                nc.tensor.matmul(
                    accs[p2][:],
                    v_sb[k][:, h * 65:(h + 1) * 65],
                    pt[:, p2 * 512:(p2 + 1) * 512],
                    start=(k == 0), stop=(k == ST - 1),
                )

        def norm_stage1(accs):
            """Evict O rows + denominators to SBUF (frees the PSUM accs for
            the next pair immediately) and start the batched reciprocal."""
            o_sb = []
            den2 = sm_pool.tile([33, 512], F32, tag="den2", name="den2")
            for p2 in range(2):
                o = sm2_pool.tile([64, 512], BF16, tag=f"o{p2}", name="osb")
                nc.vector.tensor_copy(o[:], accs[p2][0:64, :])
                o_sb.append(o)
                nc.vector.tensor_copy(den2[32 * p2:32 * p2 + 1, :],
                                      accs[p2][64:65, :])
            recip2 = sm2_pool.tile([33, 512], F32R, tag="recip2", name="recip2")
            with nc.allow_low_precision(reason="softmax denom"):
                nc.vector.reciprocal(recip2[:], den2[:])
            return (o_sb, recip2)

        def norm_apply(qb, m, st):
            """ot[m][:, qb block] = O^T * recip: PE K=1 replicate + GpSimd
            multiply (SBUF-only operands, keeps DVE free)."""
            o_sb, recip2 = st
            for p2 in range(2):
                rep = ps_w.tile([128, 512], F32, tag="pw", name="pw")
                nc.tensor.matmul(
                    rep[0:64, :], ones64_r[32 * p2:32 * p2 + 1, :],
                    recip2[32 * p2:32 * p2 + 1, :],
                    start=True, stop=True,
                )
                rep_sb = sm_pool.tile([64, 512], BF16, tag="repsb",
                                      name="repsb")
                nc.vector.tensor_copy(rep_sb[:], rep[0:64, :])
                po = 64 * p2
                nc.gpsimd.tensor_mul(
                    ot[m][po:po + 64, qb * 512:(qb + 1) * 512],
                    o_sb[p2][:], rep_sb[:])

        def yproj_i(i, ysb_holder):
            """Output projection for s-tile i; DMA when both halves done."""
            if ysb_holder[0] is None:
                ysb_holder[0] = y_pool.tile([128, D_MODEL], F32, tag="ysb", name="ysb")
            ysb = ysb_holder[0]
            for nb2 in range(2):
                ps = ps_w.tile([128, 512], F32, tag="pw", name="pw")
                for m in range(2):
                    nc.tensor.matmul(
                        ps[:],
                        ot[m][:, i * 128:(i + 1) * 128],
                        wo_r[m][:, nb2 * 512:(nb2 + 1) * 512],
                        start=(m == 0), stop=(m == 1),
                    )
                nc.vector.tensor_copy(
                    ysb[:, nb2 * 512:(nb2 + 1) * 512], ps[:])
            nc.sync.dma_start(y[i * 128:(i + 1) * 128, :], ysb[:])
            ysb_holder[0] = None

        # =============== emission schedule ===============================
        pairs = [(qb, m) for qb in range(SB) for m in range(2)]

        # lead-in: K block 0 (both halves), Q block 0
        for m in range(2):
            proj_qk_m("k", kt_sb, bk_c, 0, m)
        for m in range(2):
            proj_qk_m("q", qt, bq_c, 0, m)

        # p0: scores of pair (0,0); K blocks 1-3 + Q block 1 as PE filler
        pts_prev = []
        for k in range(ST):
            pts_prev.append(scores(0, 0, k))
            if k in (2, 4, 6):          # K blocks 1..3
                nb = k // 2
                for m in range(2):
                    proj_qk_m("k", kt_sb, bk_c, nb, m)
            elif k in (9, 12):          # Q block 1
                proj_qk_m("q", qt, bq_c, 1, 0 if k == 9 else 1)

        # windows p1..p7: scores of pair p run while the PREVIOUS pair's PV
        # drains (its pts are held; one pair of lag), then norm(prev).
        # p1 additionally weaves in the 16 V-projection chunks; later
        # windows weave in Q blocks 2-3 and the output projection.
        def fillers_for(p, k, yh):
            qb = pairs[p][0]
            if p in (2, 4) and k in (5, 11):      # Q blocks 2,3
                proj_qk_m("q", qt, bq_c, qb + 1, 0 if k == 5 else 1)
            elif p in (3, 5, 7) and k >= 6 and k % 3 == 0:
                # yproj of qb-1 (4 s-tiles at k=6,9,12,15, after norm_apply)
                yproj_i((qb - 1) * 4 + (k - 6) // 3, yh)

        yh = [None]
        prev_pair = (0, 0)
        pending_norm = None      # (qb, m, stage1 state) awaiting norm_apply
        last = len(pairs) - 1
        accs_last = None
        for p in range(1, len(pairs)):
            qb, m = pairs[p]
            accs_run = [ps_acc.tile([65, 512], F32, tag=f"acc{pp}",
                                    name=f"acc{pp}") for pp in range(2)]
            pts_cur = []
            for k in range(ST):
                pts_cur.append(scores(qb, m, k))
                if k == 5 and pending_norm is not None:
                    norm_apply(*pending_norm)
                    pending_norm = None
                if p == 1:
                    v_chunk(k)
                # prev pair's PV: two-behind so the exp it reads is long done;
                # in the final window, front-load two per iter to finish early
                if p < last:
                    if k >= 2:
                        pv(prev_pair[1], k - 2, pts_prev[k - 2], accs_run)
                elif k < 8:
                    pv(prev_pair[1], 2 * k, pts_prev[2 * k], accs_run)
                    pv(prev_pair[1], 2 * k + 1, pts_prev[2 * k + 1], accs_run)
                fillers_for(p, k, yh)
                if p == last and k == 8:
                    st = norm_stage1(accs_run)
                    pending_norm = (prev_pair[0], prev_pair[1], st)
                    accs_last = [ps_acc.tile([65, 512], F32, tag=f"acc{pp}",
                                             name=f"acc{pp}")
                                 for pp in range(2)]
                if p == last and k >= 8:
                    pv(m, k - 8, pts_cur[k - 8], accs_last)
            if p < last:
                pv(prev_pair[1], 14, pts_prev[14], accs_run)
                pv(prev_pair[1], 15, pts_prev[15], accs_run)
                st = norm_stage1(accs_run)
                pending_norm = (prev_pair[0], prev_pair[1], st)
                prev_pair, pts_prev = (qb, m), pts_cur
            else:
                prev_pair, pts_prev = (qb, m), pts_cur
        # tail: drain the last pair's remaining PV, then norm + yproj(3)
        if pending_norm is not None:
            norm_apply(*pending_norm)
            pending_norm = None
        for k in range(8, ST):
            pv(prev_pair[1], k, pts_prev[k], accs_last)
        st = norm_stage1(accs_last)
        norm_apply(prev_pair[0], prev_pair[1], st)
        for i4 in range(4):
            yproj_i(3 * 4 + i4, yh)


def _get_nc():
    global _cached_nc
    if _cached_nc is None:
        _cached_nc = build_nc()
    return _cached_nc


def _make_in_maps(query, key, value, Wq, bq, Wk, bk, Wv, bv, Wo):
    """Shard + transpose + bf16-cast on host: core c = (b, hg), b = c // HG."""
    query = np.asarray(query, dtype=np.float32)
    key = np.asarray(key, dtype=np.float32)
    value = np.asarray(value, dtype=np.float32)
    Wq, Wk, Wv, Wo = (np.asarray(w, dtype=np.float32) for w in (Wq, Wk, Wv, Wo))
    bq, bk, bv = (np.asarray(b_, dtype=np.float32) for b_ in (bq, bk, bv))
    in_maps = []
    xq_t = [np.ascontiguousarray(query[b].T).astype(BF16_NP) for b in range(B)]
    xk_t = [np.ascontiguousarray(key[b].T).astype(BF16_NP) for b in range(B)]
    xv_t = [np.ascontiguousarray(value[b].T).astype(BF16_NP) for b in range(B)]
    def tile_w(WT):          # [1024, 256] -> [128, 8*256] (k-tiles packed)
        return np.ascontiguousarray(
            WT.reshape(KT, 128, DH).transpose(1, 0, 2).reshape(128, KT * DH)
        ).astype(BF16_NP)

    for c in range(N_CORES):
        b, hg = divmod(c, HG)
        hs = slice(hg * DH, (hg + 1) * DH)
        wo_tiled = np.ascontiguousarray(
            Wo[:, hs].T.reshape(2, 128, D_MODEL).transpose(1, 0, 2)
            .reshape(128, 2 * D_MODEL)).astype(BF16_NP)
        bqk_pack = np.concatenate(
            [bq[hs].reshape(2, 128).T, bk[hs].reshape(2, 128).T],
            axis=1)          # [128, 4] = bq cols | bk cols
        in_maps.append({
            "xq_t": xq_t[b],
            "xk_t": xk_t[b],
            "xv_t": xv_t[b],
            "wq_t": tile_w(Wq[hs, :].T),
            "wk_t": tile_w(Wk[hs, :].T),
            "wv_t": tile_w(Wv[hs, :].T),
            "wo_t": wo_tiled,
            "bqk": np.ascontiguousarray(bqk_pack),
            "bv": np.ascontiguousarray(bv[hs]).reshape(1, DH).astype(BF16_NP),
        })
    return in_maps


def run(inputs, trace=False, **spmd_kwargs):
    nc = _get_nc()
    in_maps = _make_in_maps(
        inputs["query"], inputs["key"], inputs["value"],
        inputs["Wq"], inputs["bq"], inputs["Wk"], inputs["bk"],
        inputs["Wv"], inputs["bv"], inputs["Wo"])
    res = run_bass_kernel_spmd(
        nc, in_maps, list(range(N_CORES)), trace=trace, **spmd_kwargs)
    bo = np.asarray(inputs["bo"], dtype=np.float32)
    out = np.empty((B, S, D_MODEL), dtype=np.float32)
    for b in range(B):
        acc = np.zeros((S, D_MODEL), dtype=np.float32)
        for hg in range(HG):
            acc += res.results[b * HG + hg]["y"]
        out[b] = acc + bo
    return out, res


def kernel(**inputs) -> np.ndarray:
    out, _ = run(inputs, trace=False)
    return out


# revision 44
# speedup vs baseline: 1.0239x; 1.0228x over previous
"""MultiHeadAttention TRN2 Bass kernel (B=2, S=2048, D=1024, H=16, d=64).

Sharding: 8 cores = 2 (batch) x 4 (head groups of 4 heads).
Each core computes, for its batch b and head slice hs (256 dims):
    K^T = (Wk[hs,:] @ x_k^T + bk)    [256, 2048]   (dh on partitions)
    Q^T likewise; V = x_v @ Wv[hs,:].T + bv        [2048, 256]  (s on partitions)
    per head pair (2m, 2m+1): S^T = K_h @ Q_h^T, the two heads' score
    matmuls occupy disjoint PE row groups (contraction 64 at partitions
    0-63 / 64-127) and different PSUM banks -> they stream concurrently.
    P^T = exp(S^T / 8)   (scores ~ N(0,1), exp is safe without max-sub)
    [O^T ; denom] = [V_h | 1]^T @ P^T   (ones column folds the softmax
                                         denominator into the PV matmul)
    O^T = O^T * (1/denom)  (reciprocal_approx_fast, PE K=1 replicate)
    y_partial = O^T.T @ Wo[:, hs].T     [2048, 1024]
Host: y[b] = sum of 4 head-group partials + bo.

Everything the PE streams is bf16 (host-side cast: halves DMA, removes
all DVE casts, 1 cycle/row matmuls). The schedule is paced by the two
hard floors: Scalar-engine exp over 16.8M score elements (~137us) and
PE matmul rows (~110us). x is DMA'd in [128,512] column chunks through
rotating pools so the first score matmul lands ~10us in; after that the
emission keeps Scalar saturated: per sk-tile the PE emits scores(k)
BEFORE pv(k-1) (software pipeline, so the PE never blocks on the exp it
is feeding), and projection/output-projection work is woven into the
per-pair PE slack.
"""

import numpy as np
import ml_dtypes

import concourse.bass as bass
import concourse.tile as tile
import concourse.mybir as mybir
from concourse import bacc
from concourse.bass_utils import run_bass_kernel_spmd

D_MODEL = 1024
NUM_HEADS = 16
HEAD_DIM = 64
B, S = 2, 2048
N_CORES = 8
HG = 4                  # head-groups
HEADS_PER_CORE = NUM_HEADS // HG        # 4
DH = HEADS_PER_CORE * HEAD_DIM          # 256 output dims per core
KT = D_MODEL // 128                     # 8 contraction tiles
ST = S // 128                           # 16 sequence tiles
SB = S // 512                           # 4 sequence blocks of 512

F32 = mybir.dt.float32
F32R = mybir.dt.float32r
BF16 = mybir.dt.bfloat16
AF = mybir.ActivationFunctionType
BF16_NP = ml_dtypes.bfloat16

_cached_nc = None


def build_nc():
    nc = bacc.Bacc("TRN2", target_bir_lowering=False, debug=False)

    xq_t = nc.declare_dram_parameter("xq_t", [D_MODEL, S], BF16, isOutput=False)
    xk_t = nc.declare_dram_parameter("xk_t", [D_MODEL, S], BF16, isOutput=False)
    xv_t = nc.declare_dram_parameter("xv_t", [D_MODEL, S], BF16, isOutput=False)
    wq_t = nc.declare_dram_parameter("wq_t", [128, KT * DH], BF16, isOutput=False)
    wk_t = nc.declare_dram_parameter("wk_t", [128, KT * DH], BF16, isOutput=False)
    wv_t = nc.declare_dram_parameter("wv_t", [128, KT * DH], BF16, isOutput=False)
    wo_t = nc.declare_dram_parameter("wo_t", [128, 2 * D_MODEL], BF16, isOutput=False)
    bqk = nc.declare_dram_parameter("bqk", [128, 4], F32, isOutput=False)
    bv = nc.declare_dram_parameter("bv", [1, DH], BF16, isOutput=False)
    y = nc.declare_dram_parameter("y", [S, D_MODEL], F32, isOutput=True)

    with tile.TileContext(nc) as tc:
        _emit(nc, tc, xq_t, xk_t, xv_t, wq_t, wk_t, wv_t, wo_t, bqk, bv, y)
    nc.compile()
    return nc


def _emit(nc, tc, xq_t, xk_t, xv_t, wq_t, wk_t, wv_t, wo_t, bqk, bv, y):
    from contextlib import ExitStack

    ctx = ExitStack()
    with ctx:
        # ---- persistent tiles -------------------------------------------
        persist = ctx.enter_context(tc.tile_pool(name="persist", bufs=1))
        qt = [persist.tile([128, S], BF16, tag=f"qt{m}", name=f"qt{m}")
              for m in range(2)]
        kt_sb = [persist.tile([128, S], BF16, tag=f"kt{m}", name=f"kt{m}")
                 for m in range(2)]
        v_sb = [persist.tile([128, HEADS_PER_CORE * 65], BF16, tag=f"v{i}",
                             name=f"v{i}") for i in range(ST)]
        ot = [persist.tile([128, S], BF16, tag=f"ot{m}", name=f"ot{m}")
              for m in range(2)]
        wo_flat = persist.tile([128, 2 * D_MODEL], BF16, tag="wof", name="wof")
        wo_r = [wo_flat[:, m * D_MODEL:(m + 1) * D_MODEL] for m in range(2)]
        ones_row = persist.tile([1, S], BF16, tag="ones")
        ones64 = persist.tile([33, 64], F32, tag="ones64")
        ones64_r = persist.tile([33, 64], F32R, tag="ones64r")
        ones_col = persist.tile([128, HEADS_PER_CORE], F32, tag="onesc")
        bqk_c = persist.tile([128, 4], F32, tag="bqk")  # bq|bk per-partition
        bq_c, bk_c = bqk_c[:, 0:2], bqk_c[:, 2:4]
        bv_r = persist.tile([1, DH], BF16, tag="bvr")
        w_flat = {n: persist.tile([128, KT * DH], BF16, tag=f"w{n}",
                                  name=f"w{n}") for n in ("k", "q", "v")}
        w_sb = {n: [w_flat[n][:, k * DH:(k + 1) * DH] for k in range(KT)]
                for n in ("k", "q", "v")}

        # ---- x pools: fast-start [128,512] chunks + bulk remainder ------
        xk_pool = ctx.enter_context(tc.tile_pool(name="xk", bufs=8))
        xkb_pool = ctx.enter_context(tc.tile_pool(name="xkb", bufs=8))
        xq_pool = ctx.enter_context(tc.tile_pool(name="xq", bufs=8))
        xqb_pool = ctx.enter_context(tc.tile_pool(name="xqb", bufs=8))
        xv_pool = ctx.enter_context(tc.tile_pool(name="xv", bufs=8))
        xc_store = {}

        def dma_chunk1(pool, key, dram, k, nb):
            t = pool.tile([128, 512], BF16, tag="xc", name="xc")
            nc.sync.dma_start(
                t[:], dram[k * 128:(k + 1) * 128, nb * 512:(nb + 1) * 512])
            xc_store[(key, k, nb)] = (t, 0)

        def dma_chunks(pool, key, dram, nb):
            for k in range(KT):
                dma_chunk1(pool, key, dram, k, nb)

        def dma_bulk(pool, key, dram, nb0, nbn, tag):
            w = (nbn - nb0) * 512
            for k in range(KT):
                t = pool.tile([128, w], BF16, tag=tag, name="xb")
                nc.sync.dma_start(
                    t[:], dram[k * 128:(k + 1) * 128,
                               nb0 * 512:nbn * 512])
                for nb in range(nb0, nbn):
                    xc_store[(key, k, nb)] = (t, (nb - nb0) * 512)

        def xs(key, k, nb):
            t, off = xc_store[(key, k, nb)]
            return t[:, off:off + 512]

        # DMA priority order == consumption order
        nc.sync.dma_start(bqk_c[:], bqk[:, :])
        nc.sync.dma_start(w_flat["k"][:], wk_t[:, :])
        for k in range(KT):
            dma_chunk1(xk_pool, "k", xk_t, k, 0)
            dma_chunk1(xq_pool, "q", xq_t, k, 0)
        nc.sync.dma_start(w_flat["q"][:], wq_t[:, :])
        dma_bulk(xkb_pool, "k", xk_t, 1, 4, "xkb")
        dma_bulk(xqb_pool, "q", xq_t, 1, 4, "xqb")
        nc.sync.dma_start(w_flat["v"][:], wv_t[:, :])
        nc.sync.dma_start(bv_r[:], bv[:])
        dma_bulk(xv_pool, "v", xv_t, 0, 4, "xvb")
        nc.sync.dma_start(wo_flat[:], wo_t[:, :])

        # ---- pipelined-body pools ---------------------------------------
        ps_s = ctx.enter_context(
            tc.tile_pool(name="pss", bufs=2, space="PSUM"))      # 4 banks
        ps_acc = ctx.enter_context(
            tc.tile_pool(name="psacc", bufs=1, space="PSUM"))    # 2 banks
        ps_w = ctx.enter_context(
            tc.tile_pool(name="psw", bufs=2, space="PSUM"))      # 2 banks
        pt_pool = ctx.enter_context(tc.tile_pool(name="pt", bufs=19))
        sm_pool = ctx.enter_context(tc.tile_pool(name="small", bufs=1))
        sm2_pool = ctx.enter_context(tc.tile_pool(name="small2", bufs=2))
        y_pool = ctx.enter_context(tc.tile_pool(name="ysb", bufs=2))

        # constants
        nc.gpsimd.memset(ones_row[:], 1.0)
        nc.vector.memset(ones64[:], 1.0)
        nc.vector.tensor_copy(ones64_r[:], ones64[:])
        nc.vector.memset(ones_col[:], 1.0)

        # ---- building blocks --------------------------------------------
        def proj_qk_m(name, dst, bias_c, nb, m):
            """Project one (512-col, m-half) block of Q^T or K^T."""
            ps = ps_w.tile([128, 512], F32, tag="pw", name="pw")
            for k in range(KT):
                nc.tensor.matmul(
                    ps[:],
                    w_sb[name][k][:, m * 128:(m + 1) * 128],
                    xs(name, k, nb),
                    start=(k == 0), stop=(k == KT - 1),
                )
            nc.scalar.activation(
                dst[m][:, nb * 512:(nb + 1) * 512], ps[:],
                AF.Identity, bias=bias_c[:, m:m + 1])

        def v_chunk(i):
            """Project V for s-tile i into v_sb[i] (+ ones column). One
            accumulation group per PSUM tile: interleaved groups sharing a
            bank clobber each other's has_written state."""
            nb, col = divmod(i, 4)
            ps = ps_w.tile([128, 512], F32, tag="pw", name="pw")
            for k in range(KT):
                nc.tensor.matmul(
                    ps[:, 0:256],
                    xs("v", k, nb)[:, col * 128:(col + 1) * 128],
                    w_sb["v"][k][:],
                    start=(k == 0), stop=False,
                )
            nc.tensor.matmul(
                ps[:, 0:256],
                ones_row[0:1, i * 128:(i + 1) * 128],
                bv_r[0:1, :],
                start=False, stop=True,
            )
            src = ps[:, 0:256].rearrange("p (h c) -> p h c", c=64)
            vv = v_sb[i].rearrange("p (h c) -> p h c", c=65)
            nc.vector.tensor_copy(vv[:, :, 0:64], src)
            nc.vector.tensor_copy(vv[:, :, 64], ones_col[:])

        def scores(qb, m, k):
            """Score pair (heads 2m,2m+1), sk-tile k, sq-block qb. The two
            K=64 matmuls use disjoint PE row groups + PSUM banks and stream
            concurrently. Returns the exp'd bf16 tile."""
            ss = ps_s.tile([128, 1024], F32, tag="ss", name="ss")
            for p2 in range(2):
                po = 64 * p2
                nc.tensor.matmul(
                    ss[:, p2 * 512:(p2 + 1) * 512],
                    kt_sb[m][po:po + 64, k * 128:(k + 1) * 128],
                    qt[m][po:po + 64, qb * 512:(qb + 1) * 512],
                    start=True, stop=True,
                )
            pt = pt_pool.tile([128, 1024], BF16, tag="pt", name="pt")
            nc.scalar.activation(
                pt[:], ss[:], AF.Exp, scale=1.0 / float(np.sqrt(HEAD_DIM)))
            return pt

        def pv(m, k, pt, accs):
            for p2 in range(2):
                h = 2 * m + p2
                nc.tensor.matmul(
                    accs[p2][:],
                    v_sb[k][:, h * 65:(h + 1) * 65],
                    pt[:, p2 * 512:(p2 + 1) * 512],
                    start=(k == 0), stop=(k == ST - 1),
                )

        def norm_stage1(accs):
            """Evict O rows + denominators to SBUF (frees the PSUM accs for
            the next pair immediately) and start the batched reciprocal."""
            o_sb = []
            den2 = sm_pool.tile([33, 512], F32, tag="den2", name="den2")
            for p2 in range(2):
                o = sm2_pool.tile([64, 512], BF16, tag=f"o{p2}", name="osb")
                nc.vector.tensor_copy(o[:], accs[p2][0:64, :])
                o_sb.append(o)
                nc.vector.tensor_copy(den2[32 * p2:32 * p2 + 1, :],
                                      accs[p2][64:65, :])
            recip2 = sm2_pool.tile([33, 512], F32R, tag="recip2", name="recip2")
            with nc.allow_low_precision(reason="softmax denom"):
                nc.vector.reciprocal(recip2[:], den2[:])
            return (o_sb, recip2)

        def norm_apply(qb, m, st):
            """ot[m][:, qb block] = O^T * recip: PE K=1 replicate + GpSimd
            multiply (SBUF-only operands, keeps DVE free)."""
            o_sb, recip2 = st
            for p2 in range(2):
                rep = ps_w.tile([128, 512], F32, tag="pw", name="pw")
                nc.tensor.matmul(
                    rep[0:64, :], ones64_r[32 * p2:32 * p2 + 1, :],
                    recip2[32 * p2:32 * p2 + 1, :],
                    start=True, stop=True,
                )
                rep_sb = sm_pool.tile([64, 512], BF16, tag="repsb",
                                      name="repsb")
                nc.vector.tensor_copy(rep_sb[:], rep[0:64, :])
                po = 64 * p2
                nc.gpsimd.tensor_mul(
                    ot[m][po:po + 64, qb * 512:(qb + 1) * 512],
                    o_sb[p2][:], rep_sb[:])

        def yproj_i(i, ysb_holder):
            """Output projection for s-tile i; DMA when both halves done."""
            if ysb_holder[0] is None:
                ysb_holder[0] = y_pool.tile([128, D_MODEL], F32, tag="ysb", name="ysb")
            ysb = ysb_holder[0]
            for nb2 in range(2):
                ps = ps_w.tile([128, 512], F32, tag="pw", name="pw")
                for m in range(2):
                    nc.tensor.matmul(
                        ps[:],
                        ot[m][:, i * 128:(i + 1) * 128],
                        wo_r[m][:, nb2 * 512:(nb2 + 1) * 512],
                        start=(m == 0), stop=(m == 1),
                    )
                nc.vector.tensor_copy(
                    ysb[:, nb2 * 512:(nb2 + 1) * 512], ps[:])
            nc.sync.dma_start(y[i * 128:(i + 1) * 128, :], ysb[:])
            ysb_holder[0] = None

        # =============== emission schedule ===============================
        pairs = [(qb, m) for qb in range(SB) for m in range(2)]

        # lead-in: K block 0 (both halves), Q block 0
        for m in range(2):
            proj_qk_m("k", kt_sb, bk_c, 0, m)
        for m in range(2):
            proj_qk_m("q", qt, bq_c, 0, m)

        # p0: scores of pair (0,0); K blocks 1-3 + Q block 1 as PE filler
        pts_prev = []
        for k in range(ST):
            pts_prev.append(scores(0, 0, k))
            if k in (2, 4, 6):          # K blocks 1..3
                nb = k // 2
                for m in range(2):
                    proj_qk_m("k", kt_sb, bk_c, nb, m)
            elif k in (9, 12):          # Q block 1
                proj_qk_m("q", qt, bq_c, 1, 0 if k == 9 else 1)

        # windows p1..p7: scores of pair p run while the PREVIOUS pair's PV
        # drains (its pts are held; one pair of lag), then norm(prev).
        # p1 additionally weaves in the 16 V-projection chunks; later
        # windows weave in Q blocks 2-3 and the output projection.
        def fillers_for(p, k, yh):
            qb = pairs[p][0]
            if p in (2, 4) and k in (5, 11):      # Q blocks 2,3
                proj_qk_m("q", qt, bq_c, qb + 1, 0 if k == 5 else 1)
            elif p in (3, 5, 7) and k >= 6 and k % 3 == 0:
                # yproj of qb-1 (4 s-tiles at k=6,9,12,15, after norm_apply)
                yproj_i((qb - 1) * 4 + (k - 6) // 3, yh)

        yh = [None]
        prev_pair = (0, 0)
        pending_norm = None      # (qb, m, stage1 state) awaiting norm_apply
        last = len(pairs) - 1
        accs_last = None
        for p in range(1, len(pairs)):
            qb, m = pairs[p]
            accs_run = [ps_acc.tile([65, 512], F32, tag=f"acc{pp}",
                                    name=f"acc{pp}") for pp in range(2)]
            pts_cur = []
            for k in range(ST):
                pts_cur.append(scores(qb, m, k))
                if k == 5 and pending_norm is not None:
                    norm_apply(*pending_norm)
                    pending_norm = None
                if p == 1:
                    v_chunk(k)
                # prev pair's PV: two-behind so the exp it reads is long done;
                # in the final window, front-load two per iter to finish early
                if p < last:
                    if k >= 1:
                        pv(prev_pair[1], k - 1, pts_prev[k - 1], accs_run)
                elif k < 8:
                    pv(prev_pair[1], 2 * k, pts_prev[2 * k], accs_run)
                    pv(prev_pair[1], 2 * k + 1, pts_prev[2 * k + 1], accs_run)
                fillers_for(p, k, yh)
                if p == last and k == 8:
                    st = norm_stage1(accs_run)
                    pending_norm = (prev_pair[0], prev_pair[1], st)
                    accs_last = [ps_acc.tile([65, 512], F32, tag=f"acc{pp}",
                                             name=f"acc{pp}")
                                 for pp in range(2)]
                if p == last and k >= 8:
                    pv(m, k - 8, pts_cur[k - 8], accs_last)
            if p < last:
                pv(prev_pair[1], 15, pts_prev[15], accs_run)
                st = norm_stage1(accs_run)
                pending_norm = (prev_pair[0], prev_pair[1], st)
                prev_pair, pts_prev = (qb, m), pts_cur
            else:
                prev_pair, pts_prev = (qb, m), pts_cur
        # tail: drain the last pair's remaining PV, then norm + yproj(3)
        if pending_norm is not None:
            norm_apply(*pending_norm)
            pending_norm = None
        for k in range(8, ST):
            pv(prev_pair[1], k, pts_prev[k], accs_last)
        st = norm_stage1(accs_last)
        norm_apply(prev_pair[0], prev_pair[1], st)
        for i4 in range(4):
            yproj_i(3 * 4 + i4, yh)


def _get_nc():
    global _cached_nc
    if _cached_nc is None:
        _cached_nc = build_nc()
    return _cached_nc


def _make_in_maps(query, key, value, Wq, bq, Wk, bk, Wv, bv, Wo):
    """Shard + transpose + bf16-cast on host: core c = (b, hg), b = c // HG."""
    query = np.asarray(query, dtype=np.float32)
    key = np.asarray(key, dtype=np.float32)
    value = np.asarray(value, dtype=np.float32)
    Wq, Wk, Wv, Wo = (np.asarray(w, dtype=np.float32) for w in (Wq, Wk, Wv, Wo))
    bq, bk, bv = (np.asarray(b_, dtype=np.float32) for b_ in (bq, bk, bv))
    in_maps = []
    xq_t = [np.ascontiguousarray(query[b].T).astype(BF16_NP) for b in range(B)]
    xk_t = [np.ascontiguousarray(key[b].T).astype(BF16_NP) for b in range(B)]
    xv_t = [np.ascontiguousarray(value[b].T).astype(BF16_NP) for b in range(B)]
    def tile_w(WT):          # [1024, 256] -> [128, 8*256] (k-tiles packed)
        return np.ascontiguousarray(
            WT.reshape(KT, 128, DH).transpose(1, 0, 2).reshape(128, KT * DH)
        ).astype(BF16_NP)

    for c in range(N_CORES):
        b, hg = divmod(c, HG)
        hs = slice(hg * DH, (hg + 1) * DH)
        wo_tiled = np.ascontiguousarray(
            Wo[:, hs].T.reshape(2, 128, D_MODEL).transpose(1, 0, 2)
            .reshape(128, 2 * D_MODEL)).astype(BF16_NP)
        bqk_pack = np.concatenate(
            [bq[hs].reshape(2, 128).T, bk[hs].reshape(2, 128).T],
            axis=1)          # [128, 4] = bq cols | bk cols
        in_maps.append({
            "xq_t": xq_t[b],
            "xk_t": xk_t[b],
            "xv_t": xv_t[b],
            "wq_t": tile_w(Wq[hs, :].T),
            "wk_t": tile_w(Wk[hs, :].T),
            "wv_t": tile_w(Wv[hs, :].T),
            "wo_t": wo_tiled,
            "bqk": np.ascontiguousarray(bqk_pack),
            "bv": np.ascontiguousarray(bv[hs]).reshape(1, DH).astype(BF16_NP),
        })
    return in_maps


def run(inputs, trace=False, **spmd_kwargs):
    nc = _get_nc()
    in_maps = _make_in_maps(
        inputs["query"], inputs["key"], inputs["value"],
        inputs["Wq"], inputs["bq"], inputs["Wk"], inputs["bk"],
        inputs["Wv"], inputs["bv"], inputs["Wo"])
    res = run_bass_kernel_spmd(
        nc, in_maps, list(range(N_CORES)), trace=trace, **spmd_kwargs)
    bo = np.asarray(inputs["bo"], dtype=np.float32)
    out = np.empty((B, S, D_MODEL), dtype=np.float32)
    for b in range(B):
        acc = np.zeros((S, D_MODEL), dtype=np.float32)
        for hg in range(HG):
            acc += res.results[b * HG + hg]["y"]
        out[b] = acc + bo
    return out, res


def kernel(**inputs) -> np.ndarray:
    out, _ = run(inputs, trace=False)
    return out


# revision 47
# speedup vs baseline: 1.0476x; 1.0231x over previous
"""MultiHeadAttention TRN2 Bass kernel (B=2, S=2048, D=1024, H=16, d=64).

Sharding: 8 cores = 2 (batch) x 4 (head groups of 4 heads).
Each core computes, for its batch b and head slice hs (256 dims):
    K^T = (Wk[hs,:] @ x_k^T + bk)    [256, 2048]   (dh on partitions)
    Q^T likewise; V = x_v @ Wv[hs,:].T + bv        [2048, 256]  (s on partitions)
    per head pair (2m, 2m+1): S^T = K_h @ Q_h^T, the two heads' score
    matmuls occupy disjoint PE row groups (contraction 64 at partitions
    0-63 / 64-127) and different PSUM banks -> they stream concurrently.
    P^T = exp(S^T / 8)   (scores ~ N(0,1), exp is safe without max-sub)
    [O^T ; denom] = [V_h | 1]^T @ P^T   (ones column folds the softmax
                                         denominator into the PV matmul)
    O^T = O^T * (1/denom)  (reciprocal_approx_fast, PE K=1 replicate)
    y_partial = O^T.T @ Wo[:, hs].T     [2048, 1024]
Host: y[b] = sum of 4 head-group partials + bo.

Everything the PE streams is bf16 (host-side cast: halves DMA, removes
all DVE casts, 1 cycle/row matmuls). The schedule is paced by the two
hard floors: Scalar-engine exp over 16.8M score elements (~137us) and
PE matmul rows (~110us). x is DMA'd in [128,512] column chunks through
rotating pools so the first score matmul lands ~10us in; after that the
emission keeps Scalar saturated: per sk-tile the PE emits scores(k)
BEFORE pv(k-1) (software pipeline, so the PE never blocks on the exp it
is feeding), and projection/output-projection work is woven into the
per-pair PE slack.
"""

import numpy as np
import ml_dtypes

import concourse.bass as bass
import concourse.tile as tile
import concourse.mybir as mybir
from concourse import bacc
from concourse.bass_utils import run_bass_kernel_spmd

D_MODEL = 1024
NUM_HEADS = 16
HEAD_DIM = 64
B, S = 2, 2048
N_CORES = 8
HG = 4                  # head-groups
HEADS_PER_CORE = NUM_HEADS // HG        # 4
DH = HEADS_PER_CORE * HEAD_DIM          # 256 output dims per core
KT = D_MODEL // 128                     # 8 contraction tiles
ST = S // 128                           # 16 sequence tiles
SB = S // 512                           # 4 sequence blocks of 512

F32 = mybir.dt.float32
F32R = mybir.dt.float32r
BF16 = mybir.dt.bfloat16
AF = mybir.ActivationFunctionType
BF16_NP = ml_dtypes.bfloat16

_cached_nc = None


def build_nc():
    nc = bacc.Bacc("TRN2", target_bir_lowering=False, debug=False)

    xq_t = nc.declare_dram_parameter("xq_t", [D_MODEL, S], BF16, isOutput=False)
    xk_t = nc.declare_dram_parameter("xk_t", [D_MODEL, S], BF16, isOutput=False)
    xv_t = nc.declare_dram_parameter("xv_t", [D_MODEL, S], BF16, isOutput=False)
    wq_t = nc.declare_dram_parameter("wq_t", [128, KT * DH], BF16, isOutput=False)
    wk_t = nc.declare_dram_parameter("wk_t", [128, KT * DH], BF16, isOutput=False)
    wv_t = nc.declare_dram_parameter("wv_t", [128, KT * DH], BF16, isOutput=False)
    wo_t = nc.declare_dram_parameter("wo_t", [128, 2 * D_MODEL], BF16, isOutput=False)
    bqk = nc.declare_dram_parameter("bqk", [128, 4], F32, isOutput=False)
    bv = nc.declare_dram_parameter("bv", [1, DH], BF16, isOutput=False)
    y = nc.declare_dram_parameter("y", [S, D_MODEL], F32, isOutput=True)

    with tile.TileContext(nc) as tc:
        _emit(nc, tc, xq_t, xk_t, xv_t, wq_t, wk_t, wv_t, wo_t, bqk, bv, y)
    nc.compile()
    return nc


def _emit(nc, tc, xq_t, xk_t, xv_t, wq_t, wk_t, wv_t, wo_t, bqk, bv, y):
    from contextlib import ExitStack

    ctx = ExitStack()
    with ctx:
        # ---- persistent tiles -------------------------------------------
        persist = ctx.enter_context(tc.tile_pool(name="persist", bufs=1))
        qt = [persist.tile([128, S], BF16, tag=f"qt{m}", name=f"qt{m}")
              for m in range(2)]
        kt_sb = [persist.tile([128, S], BF16, tag=f"kt{m}", name=f"kt{m}")
                 for m in range(2)]
        v_sb = [persist.tile([128, HEADS_PER_CORE * 128], BF16, tag=f"v{i}",
                             name=f"v{i}") for i in range(ST)]
        ot = [persist.tile([128, S], BF16, tag=f"ot{m}", name=f"ot{m}")
              for m in range(2)]
        wo_flat = persist.tile([128, 2 * D_MODEL], BF16, tag="wof", name="wof")
        wo_r = [wo_flat[:, m * D_MODEL:(m + 1) * D_MODEL] for m in range(2)]
        ones_row = persist.tile([1, S], BF16, tag="ones")
        ones64 = persist.tile([33, 64], F32, tag="ones64")
        ones64_r = persist.tile([33, 64], F32R, tag="ones64r")
        ones_col = persist.tile([128, HEADS_PER_CORE], F32, tag="onesc")
        bqk_c = persist.tile([128, 4], F32, tag="bqk")  # bq|bk per-partition
        bq_c, bk_c = bqk_c[:, 0:2], bqk_c[:, 2:4]
        bv_r = persist.tile([1, DH], BF16, tag="bvr")
        w_flat = {n: persist.tile([128, KT * DH], BF16, tag=f"w{n}",
                                  name=f"w{n}") for n in ("k", "q", "v")}
        w_sb = {n: [w_flat[n][:, k * DH:(k + 1) * DH] for k in range(KT)]
                for n in ("k", "q", "v")}

        # ---- x pools: fast-start [128,512] chunks + bulk remainder ------
        xk_pool = ctx.enter_context(tc.tile_pool(name="xk", bufs=8))
        xkb_pool = ctx.enter_context(tc.tile_pool(name="xkb", bufs=8))
        xq_pool = ctx.enter_context(tc.tile_pool(name="xq", bufs=8))
        xqb_pool = ctx.enter_context(tc.tile_pool(name="xqb", bufs=8))
        xv_pool = ctx.enter_context(tc.tile_pool(name="xv", bufs=10))
        xc_store = {}

        def dma_chunk1(pool, key, dram, k, nb):
            t = pool.tile([128, 512], BF16, tag="xc", name="xc")
            nc.sync.dma_start(
                t[:], dram[k * 128:(k + 1) * 128, nb * 512:(nb + 1) * 512])
            xc_store[(key, k, nb)] = (t, 0)

        def dma_chunks(pool, key, dram, nb):
            for k in range(KT):
                dma_chunk1(pool, key, dram, k, nb)

        def dma_bulk(pool, key, dram, nb0, nbn, tag):
            w = (nbn - nb0) * 512
            for k in range(KT):
                t = pool.tile([128, w], BF16, tag=tag, name="xb")
                nc.sync.dma_start(
                    t[:], dram[k * 128:(k + 1) * 128,
                               nb0 * 512:nbn * 512])
                for nb in range(nb0, nbn):
                    xc_store[(key, k, nb)] = (t, (nb - nb0) * 512)

        def xs(key, k, nb):
            t, off = xc_store[(key, k, nb)]
            return t[:, off:off + 512]

        # DMA priority order == consumption order
        nc.sync.dma_start(bqk_c[:], bqk[:, :])
        nc.sync.dma_start(w_flat["k"][:], wk_t[:, :])
        nc.sync.dma_start(w_flat["v"][:], wv_t[:, :])
        nc.sync.dma_start(bv_r[:], bv[:])
        dma_chunks(xk_pool, "k", xk_t, 0)
        dma_bulk(xv_pool, "v", xv_t, 0, 2, "xvb")
        dma_chunks(xq_pool, "q", xq_t, 0)
        nc.sync.dma_start(w_flat["q"][:], wq_t[:, :])
        dma_bulk(xv_pool, "v", xv_t, 2, 4, "xvb")
        dma_bulk(xkb_pool, "k", xk_t, 1, 4, "xkb")
        dma_bulk(xqb_pool, "q", xq_t, 1, 4, "xqb")
        nc.sync.dma_start(wo_flat[:], wo_t[:, :])

        # ---- pipelined-body pools ---------------------------------------
        ps_s = ctx.enter_context(
            tc.tile_pool(name="pss", bufs=2, space="PSUM"))      # 4 banks
        ps_acc = ctx.enter_context(
            tc.tile_pool(name="psacc", bufs=1, space="PSUM"))    # 2 banks
        ps_w = ctx.enter_context(
            tc.tile_pool(name="psw", bufs=2, space="PSUM"))      # 2 banks
        pt_pool = ctx.enter_context(tc.tile_pool(name="pt", bufs=18))
        sm_pool = ctx.enter_context(tc.tile_pool(name="small", bufs=1))
        sm2_pool = ctx.enter_context(tc.tile_pool(name="small2", bufs=2))
        y_pool = ctx.enter_context(tc.tile_pool(name="ysb", bufs=2))

        # constants
        for i in range(ST):
            nc.gpsimd.memset(v_sb[i][:], 0.0)
        nc.gpsimd.memset(ones_row[:], 1.0)
        nc.vector.memset(ones64[:], 1.0)
        nc.vector.tensor_copy(ones64_r[:], ones64[:])
        nc.vector.memset(ones_col[:], 1.0)

        # ---- building blocks --------------------------------------------
        def proj_qk_m(name, dst, bias_c, nb, m):
            """Project one (512-col, m-half) block of Q^T or K^T."""
            ps = ps_w.tile([128, 512], F32, tag="pw", name="pw")
            for k in range(KT):
                nc.tensor.matmul(
                    ps[:],
                    w_sb[name][k][:, m * 128:(m + 1) * 128],
                    xs(name, k, nb),
                    start=(k == 0), stop=(k == KT - 1),
                )
            nc.vector.tensor_scalar_add(
                dst[m][:, nb * 512:(nb + 1) * 512], ps[:],
                bias_c[:, m:m + 1])

        def v_chunk(i):
            """Project V for s-tile i into v_sb[i] (+ ones column). One
            accumulation group per PSUM tile: interleaved groups sharing a
            bank clobber each other's has_written state."""
            nb, col = divmod(i, 4)
            ps = ps_w.tile([128, 512], F32, tag="pw", name="pw")
            for k in range(KT):
                nc.tensor.matmul(
                    ps[:, 0:256],
                    xs("v", k, nb)[:, col * 128:(col + 1) * 128],
                    w_sb["v"][k][:],
                    start=(k == 0), stop=False,
                )
            nc.tensor.matmul(
                ps[:, 0:256],
                ones_row[0:1, i * 128:(i + 1) * 128],
                bv_r[0:1, :],
                start=False, stop=True,
            )
            src = ps[:, 0:256].rearrange("p (h c) -> p h c", c=64)
            vv = v_sb[i].rearrange("p (h c) -> p h c", c=128)
            nc.vector.tensor_copy(vv[:, :, 0:64], src)
            nc.vector.tensor_copy(vv[:, :, 64], ones_col[:])

        def scores(qb, m, k):
            """Score pair (heads 2m,2m+1), sk-tile k, sq-block qb. The two
            K=64 matmuls use disjoint PE row groups + PSUM banks and stream
            concurrently. Returns the exp'd bf16 tile."""
            ss = ps_s.tile([128, 1024], F32, tag="ss", name="ss")
            for p2 in range(2):
                po = 64 * p2
                nc.tensor.matmul(
                    ss[:, p2 * 512:(p2 + 1) * 512],
                    kt_sb[m][po:po + 64, k * 128:(k + 1) * 128],
                    qt[m][po:po + 64, qb * 512:(qb + 1) * 512],
                    start=True, stop=True,
                )
            pt = pt_pool.tile([128, 1024], BF16, tag="pt", name="pt")
            nc.scalar.activation(
                pt[:], ss[:], AF.Exp, scale=1.0 / float(np.sqrt(HEAD_DIM)))
            return pt

        def pv(m, k, pt, accs):
            for p2 in range(2):
                h = 2 * m + p2
                nc.tensor.matmul(
                    accs[p2][:],
                    v_sb[k][:, h * 128:(h + 1) * 128],
                    pt[:, p2 * 512:(p2 + 1) * 512],
                    start=(k == 0), stop=(k == ST - 1),
                )

        def norm_stage1(accs):
            """Evict O rows + denominators to SBUF (frees the PSUM accs for
            the next pair immediately) and start the batched reciprocal."""
            o_sb = []
            den2 = sm_pool.tile([33, 512], F32, tag="den2", name="den2")
            for p2 in range(2):
                o = sm2_pool.tile([64, 512], BF16, tag=f"o{p2}", name="osb")
                nc.vector.tensor_copy(o[:], accs[p2][0:64, :])
                o_sb.append(o)
                nc.vector.tensor_copy(den2[32 * p2:32 * p2 + 1, :],
                                      accs[p2][64:65, :])
            recip2 = sm2_pool.tile([33, 512], F32R, tag="recip2", name="recip2")
            with nc.allow_low_precision(reason="softmax denom"):
                nc.vector.reciprocal(recip2[:], den2[:])
            return (o_sb, recip2)

        def norm_apply(qb, m, st):
            """ot[m][:, qb block] = O^T * recip: PE K=1 replicate + GpSimd
            multiply (SBUF-only operands, keeps DVE free)."""
            o_sb, recip2 = st
            for p2 in range(2):
                rep = ps_w.tile([128, 512], F32, tag="pw", name="pw")
                nc.tensor.matmul(
                    rep[0:64, :], ones64_r[32 * p2:32 * p2 + 1, :],
                    recip2[32 * p2:32 * p2 + 1, :],
                    start=True, stop=True,
                )
                rep_sb = sm_pool.tile([64, 512], BF16, tag="repsb",
                                      name="repsb")
                nc.vector.tensor_copy(rep_sb[:], rep[0:64, :])
                po = 64 * p2
                nc.gpsimd.tensor_mul(
                    ot[m][po:po + 64, qb * 512:(qb + 1) * 512],
                    o_sb[p2][:], rep_sb[:])

        def yproj_i(i, ysb_holder):
            """Output projection for s-tile i; DMA when both halves done."""
            if ysb_holder[0] is None:
                ysb_holder[0] = y_pool.tile([128, D_MODEL], F32, tag="ysb", name="ysb")
            ysb = ysb_holder[0]
            for nb2 in range(2):
                ps = ps_w.tile([128, 512], F32, tag="pw", name="pw")
                for m in range(2):
                    nc.tensor.matmul(
                        ps[:],
                        ot[m][:, i * 128:(i + 1) * 128],
                        wo_r[m][:, nb2 * 512:(nb2 + 1) * 512],
                        start=(m == 0), stop=(m == 1),
                    )
                nc.vector.tensor_copy(
                    ysb[:, nb2 * 512:(nb2 + 1) * 512], ps[:])
            nc.sync.dma_start(y[i * 128:(i + 1) * 128, :], ysb[:])
            ysb_holder[0] = None

        # =============== emission schedule ===============================
        # Uniform half-shifted PV pipeline. The DMA-bound lead projects
        # K/Q block 0 and ALL of V (the V matmuls soak up the xv DMA wait).
        # Window p then emits scores of pair P_p; the front half (k=0..7)
        # drains the back half of the previous pair's PV, k==8 stages the
        # previous pair's norm and swaps the PSUM accumulators, the back
        # half (k=8..15) runs the current pair's PV tiles 0..7 (8 exps
        # behind, so every PE operand is long ready). Scalar runs pure exp
        # back-to-back; remaining projections and the output projection
        # fill fixed PE slack slots.
        pairs = [(qb, m) for qb in range(SB) for m in range(2)]
        yh = [None]

        def alloc_accs():
            return [ps_acc.tile([128, 512], F32, tag=f"acc{pp}",
                                name=f"acc{pp}") for pp in range(2)]

        proj_slots = {
            (0, 1): ("k", 0, 1), (0, 3): ("k", 0, 2), (0, 5): ("k", 0, 3),
            (0, 7): ("k", 1, 0), (0, 9): ("k", 1, 1), (0, 11): ("k", 1, 2),
            (0, 13): ("k", 1, 3),
            (1, 3): ("q", 0, 1), (2, 3): ("q", 1, 1),
            (3, 3): ("q", 0, 2), (4, 3): ("q", 1, 2),
            (5, 3): ("q", 0, 3), (6, 3): ("q", 1, 3),
        }
        yproj_slots = {
            (3, 6): 0, (3, 9): 1, (3, 12): 2, (4, 2): 3,     # yproj(0)
            (5, 6): 4, (5, 9): 5, (5, 12): 6, (6, 2): 7,     # yproj(1)
            (7, 6): 8, (7, 9): 9, (7, 12): 10,               # yproj(2) i0-2
        }

        # ---- lead: K/Q block 0 (m-half 0), Q block 0 (m-half 1), all V ----
        proj_qk_m("k", kt_sb, bk_c, 0, 0)
        for i in range(ST):
            v_chunk(i)
        proj_qk_m("q", qt, bq_c, 0, 0)
        proj_qk_m("q", qt, bq_c, 0, 1)

        pts_prev = None
        carry = None            # accs being filled by the in-flight pair
        apply_q = []            # FIFO of (qb, m, stage1 state)
        for p in range(len(pairs)):
            qb, m = pairs[p]
            pts_cur = []
            for k in range(ST):
                pts_cur.append(scores(qb, m, k))
                if k == 5 and apply_q:
                    norm_apply(*apply_q.pop(0))
                if p >= 1 and k < 8:
                    pv(pairs[p - 1][1], 8 + k, pts_prev[8 + k], carry)
                if k == 8:
                    if p >= 1:
                        st = norm_stage1(carry)
                        apply_q.append((pairs[p - 1][0], pairs[p - 1][1], st))
                    carry = alloc_accs()
                if k >= 8:
                    pv(m, k - 8, pts_cur[k - 8], carry)
                if (p, k) in proj_slots:
                    nm, pm, pnb = proj_slots[(p, k)]
                    proj_qk_m(nm, kt_sb if nm == "k" else qt,
                              bk_c if nm == "k" else bq_c, pnb, pm)
                if (p, k) in yproj_slots:
                    yproj_i(yproj_slots[(p, k)], yh)
            pts_prev = pts_cur

        # ---- tail: back half of the last pair's PV, norms, yproj 2.i3/3 --
        p = len(pairs) - 1
        if apply_q:
            norm_apply(*apply_q.pop(0))
        for k in range(8):
            pv(pairs[p][1], 8 + k, pts_prev[8 + k], carry)
        yproj_i(11, yh)
        st = norm_stage1(carry)
        norm_apply(pairs[p][0], pairs[p][1], st)
        for i4 in range(4):
            yproj_i(3 * 4 + i4, yh)


def _get_nc():
    global _cached_nc
    if _cached_nc is None:
        _cached_nc = build_nc()
    return _cached_nc


def _make_in_maps(query, key, value, Wq, bq, Wk, bk, Wv, bv, Wo):
    """Shard + transpose + bf16-cast on host: core c = (b, hg), b = c // HG."""
    query = np.asarray(query, dtype=np.float32)
    key = np.asarray(key, dtype=np.float32)
    value = np.asarray(value, dtype=np.float32)
    Wq, Wk, Wv, Wo = (np.asarray(w, dtype=np.float32) for w in (Wq, Wk, Wv, Wo))
    bq, bk, bv = (np.asarray(b_, dtype=np.float32) for b_ in (bq, bk, bv))
    in_maps = []
    xq_t = [np.ascontiguousarray(query[b].T).astype(BF16_NP) for b in range(B)]
    xk_t = [np.ascontiguousarray(key[b].T).astype(BF16_NP) for b in range(B)]
    xv_t = [np.ascontiguousarray(value[b].T).astype(BF16_NP) for b in range(B)]
    def tile_w(WT):          # [1024, 256] -> [128, 8*256] (k-tiles packed)
        return np.ascontiguousarray(
            WT.reshape(KT, 128, DH).transpose(1, 0, 2).reshape(128, KT * DH)
        ).astype(BF16_NP)

    for c in range(N_CORES):
        b, hg = divmod(c, HG)
        hs = slice(hg * DH, (hg + 1) * DH)
        wo_tiled = np.ascontiguousarray(
            Wo[:, hs].T.reshape(2, 128, D_MODEL).transpose(1, 0, 2)
            .reshape(128, 2 * D_MODEL)).astype(BF16_NP)
        bqk_pack = np.concatenate(
            [bq[hs].reshape(2, 128).T, bk[hs].reshape(2, 128).T],
            axis=1)          # [128, 4] = bq cols | bk cols
        in_maps.append({
            "xq_t": xq_t[b],
            "xk_t": xk_t[b],
            "xv_t": xv_t[b],
            "wq_t": tile_w(Wq[hs, :].T),
            "wk_t": tile_w(Wk[hs, :].T),
            "wv_t": tile_w(Wv[hs, :].T),
            "wo_t": wo_tiled,
            "bqk": np.ascontiguousarray(bqk_pack),
            "bv": np.ascontiguousarray(bv[hs]).reshape(1, DH).astype(BF16_NP),
        })
    return in_maps


def run(inputs, trace=False, **spmd_kwargs):
    nc = _get_nc()
    in_maps = _make_in_maps(
        inputs["query"], inputs["key"], inputs["value"],
        inputs["Wq"], inputs["bq"], inputs["Wk"], inputs["bk"],
        inputs["Wv"], inputs["bv"], inputs["Wo"])
    res = run_bass_kernel_spmd(
        nc, in_maps, list(range(N_CORES)), trace=trace, **spmd_kwargs)
    bo = np.asarray(inputs["bo"], dtype=np.float32)
    out = np.empty((B, S, D_MODEL), dtype=np.float32)
    for b in range(B):
        acc = np.zeros((S, D_MODEL), dtype=np.float32)
        for hg in range(HG):
            acc += res.results[b * HG + hg]["y"]
        out[b] = acc + bo
    return out, res


def kernel(**inputs) -> np.ndarray:
    out, _ = run(inputs, trace=False)
    return out
